# revision 9
# baseline (speedup 1.0000x reference)
"""Additive (Bahdanau) attention kernel for Trainium2, SPMD over 8 NeuronCores.

score[b,l,k] = sum_a w3[a] * tanh(qp[b,l,a] + kp[b,k,a]);  masked softmax over k
  qp = Q @ W1^T, kp = K @ W2^T

Sharding: data-parallel over batch B=8 (one batch per core), weights replicated.

Per-core layout: attention dim `a` lives on SBUF partitions (2 tiles of 128).
 - DVE builds S[a, k, l] = qpT[a, l] + kpT[a, k] via tensor_scalar_add
   (kpT column is the per-partition scalar operand).
 - ACT does tanh on [128, 16*256] tiles (instruction overhead amortized).
 - PE reduces over `a` with tiny matmuls (lhsT = tanh tile [a,128l], rhs = w3
   [a,1]) accumulating score columns in PSUM with `l` on partitions, which is
   exactly the layout the row-softmax needs.
"""

import sys

import numpy as np

if "/opt/trn_rl_repo" not in sys.path:
    sys.path.insert(0, "/opt/trn_rl_repo")

B, LQ, LK, D, A = 8, 256, 256, 512, 256
N_CORES = 8
KB = 16  # k-block size of the main sweep

_cached_nc = None


def _build():
    from contextlib import ExitStack

    import concourse.mybir as mybir
    from concourse import tile
    from concourse.bacc import Bacc
    from concourse.masks import make_identity

    FP = mybir.dt.float32
    BF = mybir.dt.bfloat16
    I32 = mybir.dt.int32
    Act = mybir.ActivationFunctionType
    Alu = mybir.AluOpType

    nc = Bacc()
    Qd = nc.declare_dram_parameter("Q", [LQ, D], FP, isOutput=False)
    Kd = nc.declare_dram_parameter("K", [LK, D], FP, isOutput=False)
    Md = nc.declare_dram_parameter("mask", [LQ, LK], I32, isOutput=False)
    W1d = nc.declare_dram_parameter("W1", [A, D], FP, isOutput=False)
    W2d = nc.declare_dram_parameter("W2", [A, D], FP, isOutput=False)
    w3d = nc.declare_dram_parameter("w3", [A], FP, isOutput=False)
    Od = nc.declare_dram_parameter("out", [LQ, LK], FP, isOutput=True)

    with tile.TileContext(nc) as tc:
        with ExitStack() as ctx:
            const = ctx.enter_context(tc.tile_pool(name="const", bufs=1))
            load = ctx.enter_context(tc.tile_pool(name="load", bufs=1))
            trans = ctx.enter_context(tc.tile_pool(name="trans", bufs=1))
            proj = ctx.enter_context(tc.tile_pool(name="proj", bufs=1))
            spool = ctx.enter_context(tc.tile_pool(name="spool", bufs=2))
            tpool = ctx.enter_context(tc.tile_pool(name="tpool", bufs=2))
            smx = ctx.enter_context(tc.tile_pool(name="smx", bufs=2))
            pp = ctx.enter_context(tc.tile_pool(name="pp", bufs=2, space="PSUM"))
            psc = ctx.enter_context(tc.tile_pool(name="psc", bufs=1, space="PSUM"))
            pscr = ctx.enter_context(tc.tile_pool(name="pscr", bufs=1, space="PSUM"))

            # PE-only scratch: written by PE "anchor" ops that absorb one
            # cross-engine dependency each (walrus allows few sync-waits per
            # LDWEIGHTS/Matmult). Never read by other engines -> no release
            # semaphores back to PE.
            scratch = pscr.tile([128, 128], FP)

            ident = const.tile([128, 128], FP)
            make_identity(nc, ident[:])
            # Anchor: sync PE on the gpsimd-built identity.
            nc.tensor.transpose(scratch[:], ident[:], ident[:])

            # w3 [256] -> [128, 2]: column j holds a-tile j. bf16 copy for matmul.
            w3_sb = const.tile([128, 2], FP)
            nc.sync.dma_start(w3_sb[:], w3d.rearrange("(j p) -> p j", p=128))
            w3_bf = const.tile([128, 2], BF)
            nc.vector.tensor_copy(w3_bf[:], w3_sb[:])

            # One DMA per tensor (one HW queue/semaphore each) so a single
            # PE anchor per source can absorb the wait.
            q_sb = load.tile([128, 2, D], FP)
            k_sb = load.tile([128, 2, D], FP)
            w1_sb = load.tile([128, 2, D], FP)
            w2_sb = load.tile([128, 2, D], FP)
            for sb, dr in ((q_sb, Qd), (k_sb, Kd), (w1_sb, W1d), (w2_sb, W2d)):
                nc.sync.dma_start(
                    sb[:], dr.rearrange("(i p) d -> p i d", p=128)
                )

            # Transpose all four to put the contraction dim d on partitions:
            # dst layout [d 128][db 4][src-row 256].
            qT = trans.tile([128, 4, 256], FP)
            kT = trans.tile([128, 4, 256], FP)
            w1T = trans.tile([128, 4, 256], FP)
            w2T = trans.tile([128, 4, 256], FP)
            for src, dst in ((q_sb, qT), (k_sb, kT), (w1_sb, w1T), (w2_sb, w2T)):
                # Anchor: absorb this source tile's DMA wait so the real
                # transposes below only wait on psum-slot releases.
                nc.tensor.transpose(scratch[:], src[:, 0, 0:128], ident[:])
                for i in range(2):
                    for db in range(4):
                        pt = pp.tile([128, 128], FP)
                        nc.tensor.transpose(
                            pt[:], src[:, i, db * 128:(db + 1) * 128], ident[:]
                        )
                        nc.vector.tensor_copy(
                            dst[:, db, i * 128:(i + 1) * 128], pt[:]
                        )

            # Projections: qpT[a, l] = sum_d W1[a, d] * Q[l, d] (fp32), [a 128][at][l]
            qpT = proj.tile([128, 2, 256], FP)
            kpT = proj.tile([128, 2, 256], FP)
            for xT, wT, dst in ((qT, w1T, qpT), (kT, w2T, kpT)):
                for at in range(2):
                    pj = pp.tile([128, 256], FP)
                    for db in range(4):
                        nc.tensor.matmul(
                            pj[:],
                            wT[:, db, at * 128:(at + 1) * 128],
                            xT[:, db, :],
                            start=(db == 0),
                            stop=(db == 3),
                        )
                    nc.vector.tensor_copy(dst[:, at, :], pj[:])

            # mask -> additive bias: 0 where mask==1, -1e15 where mask==0
            mi = load.tile([128, 2, 256], I32)
            nc.sync.dma_start(mi[:], Md.rearrange("(i p) k -> p i k", p=128))
            mb = proj.tile([128, 2, 256], FP)
            nc.vector.tensor_copy(mb[:], mi[:])
            nc.vector.tensor_scalar(
                mb[:], mb[:], 1.0e15, -1.0e15, op0=Alu.mult, op1=Alu.add
            )

            # PSUM score accumulators, l on partitions: scores[lb][l, k]
            sc0 = psc.tile([128, 256], FP)
            sc1 = psc.tile([128, 256], FP)
            scores = [sc0, sc1]

            # Anchor: sync PE on the DVE-written w3_bf before the sweep, so
            # score matmuls only wait on the ACT-produced tanh tiles.
            nc.tensor.matmul(
                scratch[0:2, 0:2], w3_bf[:], w3_bf[:], start=True, stop=True
            )

            # Main sweep over k
            for blk in range(LK // KB):
                S = spool.tile([128, 2, KB, 256], FP)
                T = tpool.tile([128, 2, KB, 256], BF)
                for at in range(2):
                    for kk in range(KB):
                        k = blk * KB + kk
                        nc.vector.tensor_scalar_add(
                            S[:, at, kk, :], qpT[:, at, :], kpT[:, at, k:k + 1]
                        )
                    nc.scalar.activation(T[:, at], S[:, at], Act.Tanh)
                for kk in range(KB):
                    k = blk * KB + kk
                    for lb in range(2):
                        for at in range(2):
                            nc.tensor.matmul(
                                scores[lb][:, k:k + 1],
                                T[:, at, kk, lb * 128:(lb + 1) * 128],
                                w3_bf[:, at:at + 1],
                                start=(at == 0),
                                stop=(at == 1),
                            )

            # Masked softmax over k (rows = l on partitions)
            for lb in range(2):
                masked = smx.tile([128, 256], FP)
                nc.vector.tensor_add(masked[:], scores[lb][:], mb[:, lb, :])
                negmax = smx.tile([128, 1], FP)
                nc.vector.tensor_reduce(
                    negmax[:], masked[:], axis=mybir.AxisListType.X,
                    op=Alu.max, negate=True,
                )
                e = smx.tile([128, 256], FP)
                sums = smx.tile([128, 1], FP)
                nc.scalar.activation(
                    e[:], masked[:], Act.Exp,
                    bias=negmax[:], scale=1.0, accum_out=sums[:],
                )
                recip = smx.tile([128, 1], FP)
                nc.vector.reciprocal(recip[:], sums[:])
                outt = smx.tile([128, 256], FP)
                nc.vector.tensor_scalar_mul(outt[:], e[:], recip[:])
                nc.sync.dma_start(Od[lb * 128:(lb + 1) * 128, :], outt[:])

    nc.compile()
    return nc


def _get_nc():
    global _cached_nc
    if _cached_nc is None:
        _cached_nc = _build()
    return _cached_nc


def _make_in_maps(inputs):
    Q = np.ascontiguousarray(
        np.asarray(inputs["Q"], dtype=np.float32).reshape(B, LQ, D)
    )
    K = np.ascontiguousarray(
        np.asarray(inputs["K"], dtype=np.float32).reshape(B, LK, D)
    )
    mask = np.ascontiguousarray(np.asarray(inputs["mask"], dtype=np.int32))
    W1 = np.ascontiguousarray(np.asarray(inputs["W1"], dtype=np.float32))
    W2 = np.ascontiguousarray(np.asarray(inputs["W2"], dtype=np.float32))
    w3 = np.ascontiguousarray(np.asarray(inputs["w3"], dtype=np.float32))
    return [
        dict(Q=Q[i], K=K[i], mask=mask[i], W1=W1, W2=W2, w3=w3)
        for i in range(N_CORES)
    ]


def _run(inputs, trace=False, tmpdir=None):
    from concourse.bass_utils import run_bass_kernel_spmd

    nc = _get_nc()
    in_maps = _make_in_maps(inputs)
    res = run_bass_kernel_spmd(
        nc, in_maps, list(range(N_CORES)), trace=trace, tmpdir=tmpdir
    )
    out = np.stack([res.results[i]["out"] for i in range(N_CORES)], axis=0)
    return out, res


def kernel(**inputs) -> np.ndarray:
    out, _ = _run(inputs, trace=False)
    return out


# revision 13
# speedup vs baseline: 1.0325x; 1.0325x over previous
"""Additive (Bahdanau) attention kernel for Trainium2, SPMD over 8 NeuronCores.

score[b,l,k] = sum_a w3[a] * tanh(qp[b,l,a] + kp[b,k,a]);  masked softmax over k
  qp = Q @ W1^T, kp = K @ W2^T

Sharding: data-parallel over batch B=8 (one batch per core), weights replicated.

Per-core layout: attention dim `a` lives on SBUF partitions (2 tiles of 128).
 - DVE builds S[a, k, l] = qpT[a, l] + kpT[a, k] via tensor_scalar_add
   (kpT column is the per-partition scalar operand).
 - ACT does tanh on [128, 16*256] tiles (instruction overhead amortized).
 - PE reduces over `a` with tiny matmuls (lhsT = tanh tile [a,128l], rhs = w3
   [a,1]) accumulating score columns in PSUM with `l` on partitions, which is
   exactly the layout the row-softmax needs.
"""

import sys

import numpy as np

if "/opt/trn_rl_repo" not in sys.path:
    sys.path.insert(0, "/opt/trn_rl_repo")

B, LQ, LK, D, A = 8, 256, 256, 512, 256
N_CORES = 8
KB = 32   # k-block size of the main sweep
NF = 5    # k's per block handled by fused tanh(x+bias) on ACT (load balance)

_cached_nc = None


def _build():
    from contextlib import ExitStack

    import concourse.mybir as mybir
    from concourse import tile
    from concourse.bacc import Bacc
    from concourse.masks import make_identity

    FP = mybir.dt.float32
    BF = mybir.dt.bfloat16
    I32 = mybir.dt.int32
    Act = mybir.ActivationFunctionType
    Alu = mybir.AluOpType

    nc = Bacc()
    Qd = nc.declare_dram_parameter("Q", [LQ, D], FP, isOutput=False)
    Kd = nc.declare_dram_parameter("K", [LK, D], FP, isOutput=False)
    Md = nc.declare_dram_parameter("mask", [LQ, LK], I32, isOutput=False)
    W1d = nc.declare_dram_parameter("W1", [A, D], FP, isOutput=False)
    W2d = nc.declare_dram_parameter("W2", [A, D], FP, isOutput=False)
    w3d = nc.declare_dram_parameter("w3", [A], FP, isOutput=False)
    Od = nc.declare_dram_parameter("out", [LQ, LK], FP, isOutput=True)

    with tile.TileContext(nc) as tc:
        with ExitStack() as ctx:
            const = ctx.enter_context(tc.tile_pool(name="const", bufs=1))
            load = ctx.enter_context(tc.tile_pool(name="load", bufs=1))
            trans = ctx.enter_context(tc.tile_pool(name="trans", bufs=1))
            proj = ctx.enter_context(tc.tile_pool(name="proj", bufs=1))
            spool = ctx.enter_context(tc.tile_pool(name="spool", bufs=2))
            tpool = ctx.enter_context(tc.tile_pool(name="tpool", bufs=2))
            smx = ctx.enter_context(tc.tile_pool(name="smx", bufs=2))
            pp = ctx.enter_context(tc.tile_pool(name="pp", bufs=2, space="PSUM"))
            psc = ctx.enter_context(tc.tile_pool(name="psc", bufs=1, space="PSUM"))
            pscr = ctx.enter_context(tc.tile_pool(name="pscr", bufs=1, space="PSUM"))

            # PE-only scratch: written by PE "anchor" ops that absorb one
            # cross-engine dependency each (walrus allows few sync-waits per
            # LDWEIGHTS/Matmult). Never read by other engines -> no release
            # semaphores back to PE.
            scratch = pscr.tile([128, 128], FP)

            ident = const.tile([128, 128], FP)
            make_identity(nc, ident[:])
            # Anchor: sync PE on the gpsimd-built identity.
            nc.tensor.transpose(scratch[:], ident[:], ident[:])

            # w3 [256] -> [128, 2]: column j holds a-tile j. bf16 copy for matmul.
            w3_sb = const.tile([128, 2], FP)
            nc.sync.dma_start(w3_sb[:], w3d.rearrange("(j p) -> p j", p=128))
            w3_bf = const.tile([128, 2], BF)
            nc.vector.tensor_copy(w3_bf[:], w3_sb[:])

            # One DMA per tensor (one HW queue/semaphore each) so a single
            # PE anchor per source can absorb the wait.
            q_sb = load.tile([128, 2, D], FP)
            k_sb = load.tile([128, 2, D], FP)
            w1_sb = load.tile([128, 2, D], FP)
            w2_sb = load.tile([128, 2, D], FP)
            for sb, dr in ((q_sb, Qd), (k_sb, Kd), (w1_sb, W1d), (w2_sb, W2d)):
                nc.sync.dma_start(
                    sb[:], dr.rearrange("(i p) d -> p i d", p=128)
                )

            # Transpose all four to put the contraction dim d on partitions:
            # dst layout [d 128][db 4][src-row 256].
            qT = trans.tile([128, 4, 256], FP)
            kT = trans.tile([128, 4, 256], FP)
            w1T = trans.tile([128, 4, 256], FP)
            w2T = trans.tile([128, 4, 256], FP)
            for si, (src, dst) in enumerate(
                ((q_sb, qT), (k_sb, kT), (w1_sb, w1T), (w2_sb, w2T))
            ):
                # Anchor: absorb this source tile's DMA wait so the real
                # transposes below only wait on psum-slot releases.
                nc.tensor.transpose(scratch[:], src[:, 0, 0:128], ident[:])
                for i in range(2):
                    for db in range(4):
                        pt = pp.tile([128, 128], FP)
                        nc.tensor.transpose(
                            pt[:], src[:, i, db * 128:(db + 1) * 128], ident[:]
                        )
                        # split psum->sbuf copies across both streaming engines
                        eng = nc.vector if si < 2 else nc.scalar
                        if si < 2:
                            eng.tensor_copy(
                                dst[:, db, i * 128:(i + 1) * 128], pt[:]
                            )
                        else:
                            nc.scalar.copy(
                                dst[:, db, i * 128:(i + 1) * 128], pt[:]
                            )

            # Projections: qpT[a, l] = sum_d W1[a, d] * Q[l, d], [a 128][at][l].
            # bf16 copies feed the DVE adds / fused-ACT input; fp32 kpT feeds
            # the fused-ACT bias.
            qpT = proj.tile([128, 2, 256], BF)
            kpT = proj.tile([128, 2, 256], BF)
            kpF = proj.tile([128, 2, 256], FP)
            for xT, wT, dstb, dstf in (
                (qT, w1T, qpT, None), (kT, w2T, kpT, kpF)
            ):
                for at in range(2):
                    pj = pp.tile([128, 256], FP)
                    for db in range(4):
                        nc.tensor.matmul(
                            pj[:],
                            wT[:, db, at * 128:(at + 1) * 128],
                            xT[:, db, :],
                            start=(db == 0),
                            stop=(db == 3),
                        )
                    nc.vector.tensor_copy(dstb[:, at, :], pj[:])
                    if dstf is not None:
                        nc.vector.tensor_copy(dstf[:, at, :], pj[:])

            # mask -> additive bias: 0 where mask==1, -1e15 where mask==0
            mi = load.tile([128, 2, 256], I32)
            nc.sync.dma_start(mi[:], Md.rearrange("(i p) k -> p i k", p=128))
            mb = proj.tile([128, 2, 256], FP)
            nc.vector.tensor_copy(mb[:], mi[:])
            nc.vector.tensor_scalar(
                mb[:], mb[:], 1.0e15, -1.0e15, op0=Alu.mult, op1=Alu.add
            )

            # PSUM score accumulators, l on partitions: scores[lb][l, k]
            sc0 = psc.tile([128, 256], FP)
            sc1 = psc.tile([128, 256], FP)
            scores = [sc0, sc1]

            # Anchor: sync PE on the DVE-written w3_bf before the sweep, so
            # score matmuls only wait on the ACT-produced tanh tiles.
            nc.tensor.matmul(
                scratch[0:2, 0:2], w3_bf[:], w3_bf[:], start=True, stop=True
            )

            # Main sweep over k: per block, KB-NF k's go DVE-add -> batched
            # ACT tanh; the last NF k's go fused tanh(qpT + kpT[k]) on ACT
            # (per-partition bias), balancing the two streaming engines.
            NB = KB - NF
            for blk in range(LK // KB):
                S = spool.tile([128, 2, NB, 256], BF)
                T = tpool.tile([128, 2, KB, 256], BF)
                for at in range(2):
                    k0 = blk * KB
                    nc.vector.tensor_add(
                        S[:, at],
                        qpT[:, at, None, :].broadcast_to([128, NB, 256]),
                        kpT[:, at, k0:k0 + NB, None].broadcast_to(
                            [128, NB, 256]
                        ),
                    )
                    nc.scalar.activation(T[:, at, 0:NB], S[:, at], Act.Tanh)
                    for j in range(NF):
                        k = k0 + NB + j
                        nc.scalar.activation(
                            T[:, at, NB + j, :], qpT[:, at, :], Act.Tanh,
                            bias=kpF[:, at, k:k + 1],
                        )
                for kk in range(KB):
                    k = blk * KB + kk
                    for lb in range(2):
                        for at in range(2):
                            nc.tensor.matmul(
                                scores[lb][:, k:k + 1],
                                T[:, at, kk, lb * 128:(lb + 1) * 128],
                                w3_bf[:, at:at + 1],
                                start=(at == 0),
                                stop=(at == 1),
                            )

            # Masked softmax over k (rows = l on partitions)
            for lb in range(2):
                masked = smx.tile([128, 256], FP)
                nc.vector.tensor_add(masked[:], scores[lb][:], mb[:, lb, :])
                negmax = smx.tile([128, 1], FP)
                nc.vector.tensor_reduce(
                    negmax[:], masked[:], axis=mybir.AxisListType.X,
                    op=Alu.max, negate=True,
                )
                e = smx.tile([128, 256], FP)
                sums = smx.tile([128, 1], FP)
                nc.scalar.activation(
                    e[:], masked[:], Act.Exp,
                    bias=negmax[:], scale=1.0, accum_out=sums[:],
                )
                recip = smx.tile([128, 1], FP)
                nc.vector.reciprocal(recip[:], sums[:])
                outt = smx.tile([128, 256], FP)
                nc.vector.tensor_scalar_mul(outt[:], e[:], recip[:])
                nc.sync.dma_start(Od[lb * 128:(lb + 1) * 128, :], outt[:])

    nc.compile()
    return nc


def _get_nc():
    global _cached_nc
    if _cached_nc is None:
        _cached_nc = _build()
    return _cached_nc


def _make_in_maps(inputs):
    Q = np.ascontiguousarray(
        np.asarray(inputs["Q"], dtype=np.float32).reshape(B, LQ, D)
    )
    K = np.ascontiguousarray(
        np.asarray(inputs["K"], dtype=np.float32).reshape(B, LK, D)
    )
    mask = np.ascontiguousarray(np.asarray(inputs["mask"], dtype=np.int32))
    W1 = np.ascontiguousarray(np.asarray(inputs["W1"], dtype=np.float32))
    W2 = np.ascontiguousarray(np.asarray(inputs["W2"], dtype=np.float32))
    w3 = np.ascontiguousarray(np.asarray(inputs["w3"], dtype=np.float32))
    return [
        dict(Q=Q[i], K=K[i], mask=mask[i], W1=W1, W2=W2, w3=w3)
        for i in range(N_CORES)
    ]


def _run(inputs, trace=False, tmpdir=None):
    from concourse.bass_utils import run_bass_kernel_spmd

    nc = _get_nc()
    in_maps = _make_in_maps(inputs)
    res = run_bass_kernel_spmd(
        nc, in_maps, list(range(N_CORES)), trace=trace, tmpdir=tmpdir
    )
    out = np.stack([res.results[i]["out"] for i in range(N_CORES)], axis=0)
    return out, res


def kernel(**inputs) -> np.ndarray:
    out, _ = _run(inputs, trace=False)
    return out
